# revision 26
# baseline (speedup 1.0000x reference)
"""Trainium2 Bass kernel for nn_PivotGraphLearner (retrieval_knn).

Computes: weighted-cosine attention between nodes and pivots averaged over
P=8 perspectives, then keeps the top-32 values per node row (rest zero).

Math: att[n,m] = sum_p <x_n*w_p/|x_n*w_p|, y_m*w_p/|y_m*w_p|> / P
             = sum_{p,d} (x_nd/alpha_np) * (y_md*w_pd^2/(beta_mp*P))
so att = A @ B.T with K = P*D = 2048, where the W reweighting is folded
entirely into the pivot side B (host-precomputed in float64), and the node
side A = x_nd / alpha_np is built on-chip.

Precision: both operands split hi/lo into bf16 pairs; 3 accumulated matmul
passes (hi*hi + hi*lo + lo*hi) give ~1e-7 attention error (validated to be
indistinguishable from an fp32 reference through the top-k boundary).
bf16 (not fp16) because the lo terms are subnormal in fp16 and the DVE
cast flushes them; bf16 has fp32's exponent range.

Sharding: node rows split across 8 cores (6250 rows/core, padded to 6272);
pivots/weights replicated. Output rows gathered and trimmed on host.
"""

import numpy as np
import ml_dtypes

N, M, D, P = 50000, 2048, 256, 8
NCORES = 8
TILES = 49                  # 128-row node tiles per core
SHARD = TILES * 128         # 6272 padded rows per core
KC = 2 * P                  # 16 contraction chunks of 128
TOPK = 32

_cache = {}


def _build():
    import concourse.bass as bass
    from concourse import bacc
    import concourse.mybir as mybir
    import concourse.tile as tile

    f32 = mybir.dt.float32
    f16 = mybir.dt.bfloat16
    R = mybir.dt.float32r
    ts = bass.ts

    nc = bacc.Bacc("TRN2", target_bir_lowering=False)
    nodes_t = nc.dram_tensor("nodes_t", [D, SHARD], f32, kind="ExternalInput")
    b_hi = nc.dram_tensor("b_hi", [KC, 128, M], f16, kind="ExternalInput")
    b_lo = nc.dram_tensor("b_lo", [KC, 128, M], f16, kind="ExternalInput")
    ainv_in = nc.dram_tensor("ainv", [TILES, P, 128], f32, kind="ExternalInput")
    adj = nc.dram_tensor("adj", [SHARD, M], f32, kind="ExternalOutput")

    with tile.TileContext(nc) as tc:
        with (
            tc.tile_pool(name="pers", bufs=1) as pers,
            tc.tile_pool(name="work", bufs=2) as pool,
            tc.tile_pool(name="scr", bufs=3) as scr,
            tc.tile_pool(name="ps_att", bufs=2, space="PSUM") as ps_att,
        ):
            # ---- persistent: pivot matrix + weight squares ----
            bhi_sb = pers.tile([128, KC, M], f16, tag="bhi")
            blo_sb = pers.tile([128, KC, M], f16, tag="blo")
            for kc in range(KC):
                nc.sync.dma_start(bhi_sb[:, kc, :], b_hi[kc])
                nc.sync.dma_start(blo_sb[:, kc, :], b_lo[kc])
            for t in range(TILES):
                # ---- load transposed node tile (d on partitions) ----
                xT = pool.tile([128, 2, 128], f32, tag="xT")
                nc.sync.dma_start(xT[:, 0, :], nodes_t[0:128, ts(t, 128)])
                nc.sync.dma_start(xT[:, 1, :], nodes_t[128:256, ts(t, 128)])

                # ---- broadcast host-exact 1/alpha rows across partitions ----
                bc = pool.tile([128, P, 128], f32, tag="bc")
                nc.sync.dma_start(bc[:], ainv_in[t:t + 1].to_broadcast((128, P, 128)))

                # ---- node operand A = xT * ainv, split into bf16 hi/lo ----
                # layout (128, P, 128) per d-chunk c; kc = 2p + c
                a_hi = pool.tile([128, 2, P, 128], f16, tag="a_hi")
                a_lo = pool.tile([128, 2, P, 128], f16, tag="a_lo")
                for c in range(2):
                    af = scr.tile([128, P, 128], f32, tag="af")
                    xTb = xT[:, c:c + 1, :].to_broadcast((128, P, 128))
                    nc.vector.tensor_mul(af[:], xTb, bc[:])
                    nc.vector.tensor_copy(a_hi[:, c], af[:])
                    nc.vector.tensor_sub(a_lo[:, c], af[:], a_hi[:, c])

                # ---- 3-pass matmul: att = A_hi B_hi + A_hi B_lo + A_lo B_hi ----
                att_ps = ps_att.tile([128, M], f32, tag="att_ps")
                for mt in range(4):
                    sl = ts(mt, 512)
                    for kc in range(KC):
                        p, c = kc // 2, kc % 2
                        nc.tensor.matmul(att_ps[:, sl], a_hi[:, c, p, :], bhi_sb[:, kc, sl],
                                         start=(kc == 0), stop=False)
                        nc.tensor.matmul(att_ps[:, sl], a_hi[:, c, p, :], blo_sb[:, kc, sl],
                                         start=False, stop=False)
                        nc.tensor.matmul(att_ps[:, sl], a_lo[:, c, p, :], bhi_sb[:, kc, sl],
                                         start=False, stop=(kc == KC - 1))

                # ---- PSUM -> SBUF ----
                att = pool.tile([128, M], f32, tag="att")
                nc.vector.tensor_copy(att[:], att_ps[:])

                # ---- top-32 mask: 4 rounds of max8 + match_replace ----
                # imm=0.0 is safe: the 32nd-largest of 2048 zero-mean cosine
                # sims is positive w.o.p., so zapped cells never re-enter.
                work = pool.tile([128, M], f32, tag="tkwork")
                mx = pool.tile([128, 8], f32, tag="mx0")
                nc.vector.max(mx[:], att[:])
                nc.vector.match_replace(out=work[:], in_to_replace=mx[:], in_values=att[:], imm_value=0.0)
                for r in range(3):
                    mxr = pool.tile([128, 8], f32, tag=f"mx{r + 1}")
                    nc.vector.max(mxr[:], work[:])
                    nc.vector.match_replace(out=work[:], in_to_replace=mxr[:], in_values=work[:], imm_value=0.0)
                nc.vector.tensor_sub(att[:], att[:], work[:])

                nc.sync.dma_start(adj[ts(t, 128), :], att[:])

    nc.compile()
    return nc


def _round16(a64):
    hi = a64.astype(np.float32).astype(ml_dtypes.bfloat16)
    lo = (a64 - hi.astype(np.float64)).astype(np.float32).astype(ml_dtypes.bfloat16)
    return hi, lo


def kernel(nodes, pivots, weight_tensor, topk):
    assert int(topk) == TOPK
    nodes = np.asarray(nodes, dtype=np.float32)
    pivots = np.asarray(pivots, dtype=np.float32)
    W = np.asarray(weight_tensor, dtype=np.float32)

    # ---- host precompute: pivot operand B[(p,d), m], W folded in as w^2 ----
    W64 = W.astype(np.float64)
    y64 = pivots.astype(np.float64)
    yw = y64[None, :, :] * W64[:, None, :]                    # (P, M, D)
    beta = np.sqrt((yw * yw).sum(axis=2))                     # (P, M)
    B = (y64[None, :, :] * (W64 ** 2)[:, None, :]
         / beta[:, :, None] / P)                              # (P, M, D)
    # layout (KC, 128, M): kc = 2p + c, partition = d - 128c
    Bt = B.transpose(0, 2, 1).reshape(KC, 128, M)             # (p, d, m) -> chunks
    B_hi, B_lo = _round16(Bt)
    B_hi = np.ascontiguousarray(B_hi)
    B_lo = np.ascontiguousarray(B_lo)

    # ---- node shards, transposed, padded with 1.0; exact 1/alpha on host ----
    pad_total = NCORES * SHARD - N
    nodes_pad = np.concatenate(
        [nodes, np.ones((pad_total, D), dtype=np.float32)], axis=0)
    x64 = nodes_pad.astype(np.float64)
    alpha_inv = 1.0 / np.sqrt((x64 ** 2) @ (W64 ** 2).T)      # (N_pad, P)
    alpha_inv = alpha_inv.astype(np.float32)
    in_maps = []
    for c in range(NCORES):
        shard = nodes_pad[c * SHARD:(c + 1) * SHARD]
        ai = alpha_inv[c * SHARD:(c + 1) * SHARD]              # (SHARD, P)
        ai_t = np.ascontiguousarray(
            ai.reshape(TILES, 128, P).transpose(0, 2, 1))      # (TILES, P, 128)
        in_maps.append({
            "nodes_t": np.ascontiguousarray(shard.T),
            "b_hi": B_hi,
            "b_lo": B_lo,
            "ainv": ai_t,
        })

    if "nc" not in _cache:
        _cache["nc"] = _build()
    _cache["in_maps"] = in_maps
    from concourse.bass_utils import run_bass_kernel_spmd
    res = run_bass_kernel_spmd(_cache["nc"], in_maps, core_ids=list(range(NCORES)))
    out = np.concatenate([r["adj"] for r in res.results], axis=0)[:N]
    return out
